# revision 29
# baseline (speedup 1.0000x reference)
"""Trainium2 Bass kernel for CrossDecoder kNN-mining margin loss (fp8, v3).

Device mines approximate top candidates for 6000 queries over 30000
candidates (sharded 3750/core over 8 cores) via fp8 E4M3 DoubleRow matmuls:
score(q,j) = 2 q.y_j (ND data dims) + 32 b1_j + b2_j  ~=  2 q.y - (|y|^2-512).
The device emits per-16-candidate-chunk maxima; the host selects top chunks
(chunk POSITION identifies candidates), rescores exactly, computes the loss.

v3 changes (from 208us v2):
  - query dedup: only unique train_ill indices are mined (5438 -> 43 query
    tiles instead of 47), host scatters chunk maxima back.
  - scan rebalance: per query tile, DVE tensor_reduce's 1 PSUM tile (banks
    0-1) directly, ScalarE copies the other 3 tiles to SBUF fp16 TRANSPOSED
    ([w,g] -> [g,w]), so the DVE max tree runs entirely in 2x mode on
    contiguous stride-1 slices (no 1x tensor_reduce tail): 16->8->4->2->1
    pairwise tensor_tensor maxes.  DVE/j ~2.9us, Scalar/j ~2.8us, both under
    the PE's 3.5us (K=512), hiding the scan and freeing PSUM banks sooner
    (v2 stalled PE ~0.66us/j on DVE bank drains).
  - xs (candidates) DMA split into per-f-tile chunks so the first matmuls
    start before the full 2MB load lands; ~26 dummy warmup matmuls run
    during the DMA so the PE HAM clock-gate (1.2GHz cold) is released by
    the time real work starts.
"""

import os
import numpy as np
import ml_dtypes

M_, N_, D_, T_ = 2, 30000, 256, 3000
KD = M_ * D_                   # 512 contraction (data) dims
NCORES = 8
NSHARD = N_ // NCORES          # 3750
GW = 16                        # candidates per chunk
FCH = 480                      # candidate tile width (one PSUM bank, 30 groups)
NFC = 8                        # candidate tiles per core
NPAD = FCH * NFC               # 3840
NGRP = FCH // GW               # 30 chunk maxima per tile
S1 = 32.0                      # bias row 1 scale
CENTER = 512.0                 # |y|^2 centering (cancels in ranking)
QT = 128                       # queries per tile (PSUM partition dim)
QBLK = 4                       # query tiles per DMA block
XCH = 2                        # xs f-tiles per DMA chunk
F1W = 400                      # f=1 is a partial tile: 390 real cands + 10
F1G = F1W // GW                # pad, rounded to 25 chunk groups (saves 80
                               # matmul columns/pass vs padding to 480)

NKC = int(os.environ.get("KNN_NKC", "2"))   # k-tile pairs: 2 -> K=512 screen
ND = 128 * 2 * NKC - 2                      # data dims used for selection
NSEL = int(os.environ.get("KNN_NSEL", "48" if NKC == 2 else "96"))

_cache = {}


def _build_program(nqt):
    import concourse.bass as bass
    import concourse.tile as tile
    from concourse import bacc, mybir

    dt = mybir.dt
    nc = bacc.Bacc(
        "TRN2", target_bir_lowering=False, debug=False, num_devices=NCORES
    )

    nq = nqt * QT
    nblk = (nqt + QBLK - 1) // QBLK
    xq_d = nc.dram_tensor("xq", [128, 2 * NKC, nq], dt.float8e4,
                          kind="ExternalInput")
    xs_d = nc.dram_tensor("xs", [128, 2 * NKC, NPAD], dt.float8e4,
                          kind="ExternalInput")
    cand_d = nc.dram_tensor("cand", [nblk, 128, QBLK * NFC * NGRP], dt.float16,
                            kind="ExternalOutput")

    DR = mybir.MatmulPerfMode.DoubleRow

    with tile.TileContext(nc) as tc:
        with (
            tc.tile_pool(name="resident", bufs=1) as res_pool,
            tc.tile_pool(name="xq", bufs=2) as xq_pool,
            tc.tile_pool(name="cand", bufs=2) as cand_pool,
            tc.tile_pool(name="scr", bufs=2) as scr_pool,
            tc.tile_pool(name="psum", bufs=4, space=bass.MemorySpace.PSUM) as psum_pool,
        ):
            # --- resident candidates: one SBUF tile per DMA chunk, so the
            # first matmuls' dependencies resolve as soon as their own chunk
            # lands (a single tile would gate on the full 2MB load).
            xs_sbs = [res_pool.tile([128, 2 * NKC, XCH * FCH], dt.float8e4,
                                    tag=f"xs{ci}", name=f"xs{ci}")
                      for ci in range(NFC // XCH)]

            def xs_slice(kc, f):
                t = xs_sbs[f // XCH]
                fo = f % XCH
                w = F1W if f == 1 else FCH
                return t[:, 2 * kc:2 * kc + 2, fo * FCH:fo * FCH + w]

            xq_tiles = {}

            def xq_dma(blk):
                q0 = blk * QBLK * QT
                bqt = min(QBLK, nqt - blk * QBLK)
                t = xq_pool.tile([128, 2 * NKC, bqt * QT], dt.float8e4,
                                 tag="xq", name="xq_sb")
                nc.sync.dma_start(out=t[:, :, :],
                                  in_=xq_d[:, :, q0:q0 + bqt * QT])
                xq_tiles[blk] = t

            def xs_dma(ci):
                c0 = ci * XCH * FCH
                nc.sync.dma_start(out=xs_sbs[ci][:, :, :],
                                  in_=xs_d[:, :, c0:c0 + XCH * FCH])

            xs_dma(0)
            xq_dma(0)
            for ci in range(1, NFC // XCH):
                xs_dma(ci)

            def emit_tree(scr, cand_out, ntile=3):
                """DVE max tree over ntile Scalar-copied tiles: 16->8->4->2
                in 2x mode, final 2->1 a strided 1x tensor_tensor.
                scr: [128, ntile, 2, NGRP, GW]; cand_out: [128, 2*ntile, NGRP]
                """
                t1 = scr_pool.tile([128, ntile, 2, NGRP, 8], dt.float16,
                                   tag=f"t1_{ntile}")
                nc.vector.tensor_tensor(
                    t1[:, :, :, :, :], scr[:, :, :, :, 0:8],
                    scr[:, :, :, :, 8:16], mybir.AluOpType.max)
                t2 = scr_pool.tile([128, ntile, 2, NGRP, 4], dt.float16,
                                   tag=f"t2_{ntile}")
                nc.vector.tensor_tensor(
                    t2[:, :, :, :, :], t1[:, :, :, :, 0:4],
                    t1[:, :, :, :, 4:8], mybir.AluOpType.max)
                t3 = scr_pool.tile([128, ntile, 2, NGRP, 2], dt.float16,
                                   tag=f"t3_{ntile}")
                nc.vector.tensor_tensor(
                    t3[:, :, :, :, :], t2[:, :, :, :, 0:2],
                    t2[:, :, :, :, 2:4], mybir.AluOpType.max)
                nc.vector.tensor_tensor(
                    cand_out.rearrange("p (t a) g -> p t a g", t=ntile),
                    t3[:, :, :, :, 0], t3[:, :, :, :, 1],
                    mybir.AluOpType.max)

            # tree/output emission is deferred one query tile so the DVE
            # queue runs [reduce_j, tree_{j-1}, reduce_{j+1}, ...]: the
            # reduce that frees PSUM banks 0-1 for the next tile's matmuls
            # is never stuck behind a tree waiting on Scalar copies.
            pending_tree = None
            pending_cand = None

            for blk in range(nblk):
                bqt = min(QBLK, nqt - blk * QBLK)
                if blk + 1 < nblk:
                    xq_dma(blk + 1)
                xq_sb = xq_tiles.pop(blk)
                cand_sb = cand_pool.tile([128, bqt, NFC, NGRP], dt.float16,
                                         tag="cand")
                for j in range(bqt):
                    ps = [psum_pool.tile([128, 2, NGRP, GW], dt.float32,
                                         tag="ps", name=f"ps{p}",
                                         padded_shape=[None, None, 32, None])
                          for p in range(4)]

                    def mm(kc, f):
                        ng = F1G if f == 1 else NGRP
                        nc.tensor.matmul(
                            ps[f // 2][:, f % 2, 0:ng, :],
                            lhsT=xq_sb[:, 2 * kc:2 * kc + 2,
                                       j * QT:(j + 1) * QT],
                            rhs=xs_slice(kc, f),
                            start=(kc == 0),
                            stop=(kc == NKC - 1),
                            perf_mode=DR,
                        )
                    # f-major, kc-minor: each PSUM pair p completes at MM
                    # #(4p+4), so its consumer (reduce / Scalar copy) starts
                    # as early as possible and frees banks for the next tile.
                    # The two stationaries ping-pong the PE weight buffers;
                    # LDWEIGHTS (135ns) hides under each matmul (203ns).
                    # Ordinary tiles: DVE tensor_reduce's p0, Scalar copies
                    # p1..p3.  The LAST tile flips roles (Scalar copies
                    # p0..p2 with per-pair trees, DVE reduces p3 at the end)
                    # so only a ~1.1us reduce trails the final matmul.
                    last = (blk == nblk - 1) and (j == bqt - 1)
                    rset = {0, 3} if last else {0}  # the DVE-reduced pairs
                    scr = scr_pool.tile([128, 3, 2, NGRP, GW], dt.float16,
                                        tag="scr")
                    si = 0
                    for f in range(NFC):
                        for kc in range(NKC):
                            mm(kc, f)
                        if f % 2 == 1:
                            p = f // 2
                            if p == 0:
                                # split reduce: bank 1 (f=1) only has F1G
                                # valid groups; never read unwritten PSUM
                                nc.vector.tensor_reduce(
                                    cand_sb[:, j, 0:1, :],
                                    ps[0][:, 0:1, :, :],
                                    axis=mybir.AxisListType.X,
                                    op=mybir.AluOpType.max,
                                )
                                nc.vector.tensor_reduce(
                                    cand_sb[:, j, 1:2, 0:F1G],
                                    ps[0][:, 1:2, 0:F1G, :],
                                    axis=mybir.AxisListType.X,
                                    op=mybir.AluOpType.max,
                                )
                            elif p in rset:
                                nc.vector.tensor_reduce(
                                    cand_sb[:, j, 2 * p:2 * p + 2, :],
                                    ps[p][:, :, :, :],
                                    axis=mybir.AxisListType.X,
                                    op=mybir.AluOpType.max,
                                )
                            elif last:
                                nc.scalar.activation(
                                    scr[:, si, :, :, :], ps[p][:, :, :, :],
                                    mybir.ActivationFunctionType.Copy,
                                )
                                emit_tree(scr[:, si:si + 1, :, :, :],
                                          cand_sb[:, j, 2 * p:2 * p + 2, :],
                                          ntile=1)
                                si += 1
                    if pending_tree is not None:
                        emit_tree(*pending_tree)
                        pending_tree = None
                    if not last:
                        # tiles p1..p3: ScalarE contiguous copy PSUM->SBUF
                        for p in range(1, 4):
                            nc.scalar.activation(
                                scr[:, p - 1, :, :, :], ps[p][:, :, :, :],
                                mybir.ActivationFunctionType.Copy,
                            )
                        pending_tree = (scr, cand_sb[:, j, 2:8, :])
                    if pending_cand is not None:
                        nc.scalar.dma_start(out=pending_cand[0],
                                            in_=pending_cand[1])
                        pending_cand = None
                # output DMA on the Scalar queue (HWDGE) so the sync queue
                # never blocks on a cand-ready wait; deferred past the next
                # tile's copies so the wait is already satisfied.
                pending_cand = (cand_d[blk, :, :bqt * NFC * NGRP],
                                cand_sb[:, :, :, :])
            if pending_tree is not None:
                emit_tree(*pending_tree)
            if pending_cand is not None:
                nc.scalar.dma_start(out=pending_cand[0], in_=pending_cand[1])

    nc.compile()
    return nc


def _get_program(nqt):
    key = ("nc", nqt, NKC)
    if key not in _cache:
        _cache[key] = _build_program(nqt)
    return _cache[key]


def _f8(a):
    return np.clip(np.asarray(a, np.float32), -240, 240).astype(
        ml_dtypes.float8_e4m3)


# xs column -> local candidate index (-1 = pad/dead).  f=1 is the partial
# tile: columns 480:870 hold cands 480:870, 870:960 are dead; f>=2 tiles
# hold the remaining 2880 candidates.
_F1R = NSHARD - (NFC - 1) * FCH          # 390 real cands in the f=1 tile


def _col_to_loc():
    loc = np.full(NPAD, -1, np.int64)
    loc[0:FCH] = np.arange(FCH)
    loc[FCH:FCH + _F1R] = np.arange(FCH, FCH + _F1R)
    loc[2 * FCH:] = np.arange(FCH + _F1R, NSHARD)
    return loc


def _prep_inputs(X, uq, nq):
    """X: [N, 512] fp32; uq: unique query ids (len <= nq)."""
    kd = 2 * NKC * 128
    Qm = np.zeros((nq, kd), np.float32)
    nu = len(uq)
    Qm[:nu, :ND] = 2.0 * X[uq, :ND]
    Qm[:nu, ND] = S1
    Qm[:nu, ND + 1] = 1.0
    xq = np.ascontiguousarray(
        _f8(Qm).reshape(nq, 2 * NKC, 128).transpose(2, 1, 0))

    sqy = (X.astype(np.float64) ** 2).sum(1).astype(np.float32)
    bias_t = -(sqy - CENTER)
    b1 = _f8(bias_t / S1).astype(np.float32)
    b2 = _f8(bias_t - S1 * b1).astype(np.float32)

    loc = _col_to_loc()
    vm = loc >= 0
    per_core = []
    for ci in range(NCORES):
        sl = slice(ci * NSHARD, (ci + 1) * NSHARD)
        Xc = X[sl]
        Z = np.zeros((NPAD, kd), np.float32)
        Z[vm, :ND] = Xc[loc[vm], :ND]
        Z[vm, ND] = b1[sl][loc[vm]]
        Z[vm, ND + 1] = b2[sl][loc[vm]]
        Z[~vm, ND:ND + 2] = -240.0        # pad candidates rank last
        xs = np.ascontiguousarray(
            _f8(Z).reshape(NPAD, 2 * NKC, 128).transpose(2, 1, 0))
        per_core.append({"xq": xq, "xs": xs})
    return per_core


def _mine_chunkmax(in_maps, nqt, trace=False):
    from concourse.bass_utils import run_bass_kernel_spmd

    nc = _get_program(nqt)
    try:
        res = run_bass_kernel_spmd(nc, in_maps, list(range(NCORES)), trace=trace)
    except Exception:
        if not trace:
            raise
        res = run_bass_kernel_spmd(nc, in_maps, list(range(NCORES)), trace=False)
    _cache["last_result"] = res
    nblk = (nqt + QBLK - 1) // QBLK
    cores = []
    for i in range(NCORES):
        c = res.results[i]["cand"]                 # [nblk, 128, QBLK*240]
        c = c.reshape(nblk, 128, QBLK, NFC * NGRP).transpose(0, 2, 1, 3)
        cores.append(c.reshape(nblk * QBLK * 128, NFC * NGRP)[:nqt * QT])
    return np.concatenate(cores, axis=1)           # [nq, 1920]


def kernel(outlayer, c, train_ill, k):
    k = int(k)
    outlayer = np.asarray(outlayer, np.float32)
    train_ill = np.asarray(train_ill)
    X = np.ascontiguousarray(
        outlayer.transpose(1, 0, 2).reshape(N_, KD)).astype(np.float32)
    left = train_ill[:, 0].astype(np.int64)
    right = train_ill[:, 1].astype(np.int64)
    q_idx = np.concatenate([right, left])          # [2T]

    uq, inv = np.unique(q_idx, return_inverse=True)
    nqt = max(1, (len(uq) + QT - 1) // QT)
    nq = nqt * QT

    in_maps = _prep_inputs(X, uq, nq)
    cmu = _mine_chunkmax(
        in_maps, nqt, trace=bool(int(os.environ.get("KNN_TRACE", "0"))))
    cm = cmu[inv].astype(np.float32)               # [2T, 1920]

    # mask chunks the device never writes (f=1 groups >= F1G: stale SBUF)
    nch = NPAD // GW                               # 240 chunks per core
    ch = np.arange(nch)
    dead = ((ch // NGRP) == 1) & ((ch % NGRP) >= F1G)
    cm[:, np.tile(dead, NCORES)] = -np.inf

    # top-NSEL chunks per query -> candidate lists with known indices
    top_chunks = np.argpartition(-cm, NSEL, axis=1)[:, :NSEL]
    core = top_chunks // nch
    base_col = ((top_chunks % nch) // NGRP) * FCH \
        + ((top_chunks % nch) % NGRP) * GW
    loc = _col_to_loc()[
        base_col[:, :, None] + np.arange(GW)[None, None, :]]  # [2T,NSEL,16]
    valid = loc >= 0
    cand = np.where(valid, core[:, :, None] * NSHARD + np.maximum(loc, 0), 0)
    cand = cand.reshape(2 * T_, NSEL * GW)
    valid = valid.reshape(2 * T_, NSEL * GW)

    # exact rescore (fp32 gather/dot, fp64 assembly)
    nkeep = k + 1
    sq64 = (X.astype(np.float64) ** 2).sum(1)
    B_all = np.zeros((2 * T_, nkeep))
    for q0 in range(0, 2 * T_, 256):
        q1 = min(q0 + 256, 2 * T_)
        qv = X[q_idx[q0:q1]]                                   # [B, 512]
        cv = X[cand[q0:q1]]                                    # [B, C, 512]
        dot = np.matmul(cv, qv[:, :, None].astype(np.float32))[:, :, 0]
        d = (sq64[q_idx[q0:q1], None] + sq64[cand[q0:q1]]
             - 2.0 * dot.astype(np.float64))
        d = np.where(valid[q0:q1], d, np.inf)
        idx = np.argpartition(d, nkeep, axis=1)[:, :nkeep]
        g = X.astype(np.float64)[np.take_along_axis(cand[q0:q1], idx, axis=1)]
        dd = ((qv[:, None, :].astype(np.float64) - g) ** 2).sum(2)
        dd = np.where(np.take_along_axis(valid[q0:q1], idx, axis=1), dd, np.inf)
        B_all[q0:q1] = np.sort(dd, axis=1)
    B2 = B_all[:T_, 1:]            # right-query mining
    B1 = B_all[T_:, 1:]            # left-query mining

    X64 = X.astype(np.float64)
    D = ((X64[left] - X64[right]) ** 2).sum(1) + 1.0
    L1 = np.maximum(D[:, None] - B1, 0.0)
    L2 = np.maximum(D[:, None] - B2, 0.0)
    loss = (L1.mean() + L2.mean()) / 2.0
    return np.asarray(loss, dtype=np.float32)


# revision 30
# speedup vs baseline: 1.0945x; 1.0945x over previous
"""Trainium2 Bass kernel for CrossDecoder kNN-mining margin loss (fp8, v3).

Device mines approximate top candidates for 6000 queries over 30000
candidates (sharded 3750/core over 8 cores) via fp8 E4M3 DoubleRow matmuls:
score(q,j) = 2 q.y_j (ND data dims) + 32 b1_j + b2_j  ~=  2 q.y - (|y|^2-512).
The device emits per-16-candidate-chunk maxima; the host selects top chunks
(chunk POSITION identifies candidates), rescores exactly, computes the loss.

v3 changes (from 208us v2):
  - query dedup: only unique train_ill indices are mined (5438 -> 43 query
    tiles instead of 47), host scatters chunk maxima back.
  - scan rebalance: per query tile, DVE tensor_reduce's 1 PSUM tile (banks
    0-1) directly, ScalarE copies the other 3 tiles to SBUF fp16 TRANSPOSED
    ([w,g] -> [g,w]), so the DVE max tree runs entirely in 2x mode on
    contiguous stride-1 slices (no 1x tensor_reduce tail): 16->8->4->2->1
    pairwise tensor_tensor maxes.  DVE/j ~2.9us, Scalar/j ~2.8us, both under
    the PE's 3.5us (K=512), hiding the scan and freeing PSUM banks sooner
    (v2 stalled PE ~0.66us/j on DVE bank drains).
  - xs (candidates) DMA split into per-f-tile chunks so the first matmuls
    start before the full 2MB load lands; ~26 dummy warmup matmuls run
    during the DMA so the PE HAM clock-gate (1.2GHz cold) is released by
    the time real work starts.
"""

import os
import numpy as np
import ml_dtypes

M_, N_, D_, T_ = 2, 30000, 256, 3000
KD = M_ * D_                   # 512 contraction (data) dims
NCORES = 8
NSHARD = N_ // NCORES          # 3750
GW = 16                        # candidates per chunk
FCH = 480                      # candidate tile width (one PSUM bank, 30 groups)
NFC = 8                        # candidate tiles per core
NPAD = FCH * NFC               # 3840
NGRP = FCH // GW               # 30 chunk maxima per tile
S1 = 32.0                      # bias row 1 scale
CENTER = 512.0                 # |y|^2 centering (cancels in ranking)
QT = 128                       # queries per tile (PSUM partition dim)
QBLK = 4                       # query tiles per DMA block
XCH = 2                        # xs f-tiles per DMA chunk
F1W = 400                      # f=1 is a partial tile: 390 real cands + 10
F1G = F1W // GW                # pad, rounded to 25 chunk groups (saves 80
                               # matmul columns/pass vs padding to 480)

NKC = int(os.environ.get("KNN_NKC", "2"))   # k-tile pairs: 2 -> K=512 screen
ND = 128 * 2 * NKC - 2                      # data dims used for selection
NSEL = int(os.environ.get("KNN_NSEL", "48" if NKC == 2 else "96"))

_cache = {}


def _build_program(nqt):
    import concourse.bass as bass
    import concourse.tile as tile
    from concourse import bacc, mybir

    dt = mybir.dt
    nc = bacc.Bacc(
        "TRN2", target_bir_lowering=False, debug=False, num_devices=NCORES
    )

    nq = nqt * QT
    nblk = (nqt + QBLK - 1) // QBLK
    xq_d = nc.dram_tensor("xq", [128, 2 * NKC, nq], dt.float8e4,
                          kind="ExternalInput")
    xs_d = nc.dram_tensor("xs", [128, 2 * NKC, NPAD], dt.float8e4,
                          kind="ExternalInput")
    cand_d = nc.dram_tensor("cand", [nblk, 128, QBLK * NFC * NGRP], dt.float16,
                            kind="ExternalOutput")

    DR = mybir.MatmulPerfMode.DoubleRow

    with tile.TileContext(nc) as tc:
        with (
            tc.tile_pool(name="resident", bufs=1) as res_pool,
            tc.tile_pool(name="xq", bufs=2) as xq_pool,
            tc.tile_pool(name="cand", bufs=2) as cand_pool,
            tc.tile_pool(name="scr", bufs=2) as scr_pool,
            tc.tile_pool(name="psum", bufs=4, space=bass.MemorySpace.PSUM) as psum_pool,
        ):
            # --- resident candidates: one SBUF tile per DMA chunk, so the
            # first matmuls' dependencies resolve as soon as their own chunk
            # lands (a single tile would gate on the full 2MB load).
            xs_sbs = [res_pool.tile([128, 2 * NKC, XCH * FCH], dt.float8e4,
                                    tag=f"xs{ci}", name=f"xs{ci}")
                      for ci in range(NFC // XCH)]

            def xs_slice(kc, f):
                t = xs_sbs[f // XCH]
                fo = f % XCH
                w = F1W if f == 1 else FCH
                return t[:, 2 * kc:2 * kc + 2, fo * FCH:fo * FCH + w]

            xq_tiles = {}

            def xq_dma(blk):
                q0 = blk * QBLK * QT
                bqt = min(QBLK, nqt - blk * QBLK)
                t = xq_pool.tile([128, 2 * NKC, bqt * QT], dt.float8e4,
                                 tag="xq", name="xq_sb")
                nc.sync.dma_start(out=t[:, :, :],
                                  in_=xq_d[:, :, q0:q0 + bqt * QT])
                xq_tiles[blk] = t

            def xs_dma(ci):
                c0 = ci * XCH * FCH
                nc.sync.dma_start(out=xs_sbs[ci][:, :, :],
                                  in_=xs_d[:, :, c0:c0 + XCH * FCH])

            xs_dma(0)
            xq_dma(0)
            for ci in range(1, NFC // XCH):
                xs_dma(ci)

            def emit_tree(scr, cand_out, ntile=3):
                """DVE max tree over ntile Scalar-copied tiles: 16->8->4->2
                in 2x mode, final 2->1 a strided 1x tensor_tensor.
                scr: [128, ntile, 2, NGRP, GW]; cand_out: [128, 2*ntile, NGRP]
                """
                t1 = scr_pool.tile([128, ntile, 2, NGRP, 8], dt.float16,
                                   tag=f"t1_{ntile}")
                nc.vector.tensor_tensor(
                    t1[:, :, :, :, :], scr[:, :, :, :, 0:8],
                    scr[:, :, :, :, 8:16], mybir.AluOpType.max)
                t2 = scr_pool.tile([128, ntile, 2, NGRP, 4], dt.float16,
                                   tag=f"t2_{ntile}")
                nc.vector.tensor_tensor(
                    t2[:, :, :, :, :], t1[:, :, :, :, 0:4],
                    t1[:, :, :, :, 4:8], mybir.AluOpType.max)
                t3 = scr_pool.tile([128, ntile, 2, NGRP, 2], dt.float16,
                                   tag=f"t3_{ntile}")
                nc.vector.tensor_tensor(
                    t3[:, :, :, :, :], t2[:, :, :, :, 0:2],
                    t2[:, :, :, :, 2:4], mybir.AluOpType.max)
                nc.vector.tensor_tensor(
                    cand_out.rearrange("p (t a) g -> p t a g", t=ntile),
                    t3[:, :, :, :, 0], t3[:, :, :, :, 1],
                    mybir.AluOpType.max)

            # tree/output emission is deferred one query tile so the DVE
            # queue runs [reduce_j, tree_{j-1}, reduce_{j+1}, ...]: the
            # reduce that frees PSUM banks 0-1 for the next tile's matmuls
            # is never stuck behind a tree waiting on Scalar copies.
            pending_tree = None
            pending_cand = None

            for blk in range(nblk):
                bqt = min(QBLK, nqt - blk * QBLK)
                if blk + 1 < nblk:
                    xq_dma(blk + 1)
                xq_sb = xq_tiles.pop(blk)
                cand_sb = cand_pool.tile([128, bqt, NFC, NGRP], dt.float16,
                                         tag="cand")
                for j in range(bqt):
                    ps = [psum_pool.tile([128, 2, NGRP, GW], dt.float32,
                                         tag="ps", name=f"ps{p}",
                                         padded_shape=[None, None, 32, None])
                          for p in range(4)]

                    def mm(kc, f):
                        ng = F1G if f == 1 else NGRP
                        nc.tensor.matmul(
                            ps[f // 2][:, f % 2, 0:ng, :],
                            lhsT=xq_sb[:, 2 * kc:2 * kc + 2,
                                       j * QT:(j + 1) * QT],
                            rhs=xs_slice(kc, f),
                            start=(kc == 0),
                            stop=(kc == NKC - 1),
                            perf_mode=DR,
                        )
                    # f-major, kc-minor: each PSUM pair p completes at MM
                    # #(4p+4), so its consumer (reduce / Scalar copy) starts
                    # as early as possible and frees banks for the next tile.
                    # The two stationaries ping-pong the PE weight buffers;
                    # LDWEIGHTS (135ns) hides under each matmul (203ns).
                    # Ordinary tiles: DVE tensor_reduce's p0, Scalar copies
                    # p1..p3.  The LAST tile flips roles (Scalar copies
                    # p0..p2 with per-pair trees, DVE reduces p3 at the end)
                    # so only a ~1.1us reduce trails the final matmul.
                    last = (blk == nblk - 1) and (j == bqt - 1)
                    rset = {0, 3} if last else {0}  # the DVE-reduced pairs
                    scr = scr_pool.tile([128, 3, 2, NGRP, GW], dt.float16,
                                        tag="scr")
                    si = 0
                    for f in range(NFC):
                        for kc in range(NKC):
                            mm(kc, f)
                        if f % 2 == 1:
                            p = f // 2
                            if p in rset:
                                nc.vector.tensor_reduce(
                                    cand_sb[:, j, 2 * p:2 * p + 2, :],
                                    ps[p][:, :, :, :],
                                    axis=mybir.AxisListType.X,
                                    op=mybir.AluOpType.max,
                                )
                            elif last:
                                nc.scalar.activation(
                                    scr[:, si, :, :, :], ps[p][:, :, :, :],
                                    mybir.ActivationFunctionType.Copy,
                                )
                                emit_tree(scr[:, si:si + 1, :, :, :],
                                          cand_sb[:, j, 2 * p:2 * p + 2, :],
                                          ntile=1)
                                si += 1
                    if pending_tree is not None:
                        emit_tree(*pending_tree)
                        pending_tree = None
                    if not last:
                        # tiles p1..p3: ScalarE contiguous copy PSUM->SBUF
                        for p in range(1, 4):
                            nc.scalar.activation(
                                scr[:, p - 1, :, :, :], ps[p][:, :, :, :],
                                mybir.ActivationFunctionType.Copy,
                            )
                        pending_tree = (scr, cand_sb[:, j, 2:8, :])
                    if pending_cand is not None:
                        nc.scalar.dma_start(out=pending_cand[0],
                                            in_=pending_cand[1])
                        pending_cand = None
                # output DMA on the Scalar queue (HWDGE) so the sync queue
                # never blocks on a cand-ready wait; deferred past the next
                # tile's copies so the wait is already satisfied.
                pending_cand = (cand_d[blk, :, :bqt * NFC * NGRP],
                                cand_sb[:, :, :, :])
            if pending_tree is not None:
                emit_tree(*pending_tree)
            if pending_cand is not None:
                nc.scalar.dma_start(out=pending_cand[0], in_=pending_cand[1])

    nc.compile()
    return nc


def _get_program(nqt):
    key = ("nc", nqt, NKC)
    if key not in _cache:
        _cache[key] = _build_program(nqt)
    return _cache[key]


def _f8(a):
    return np.clip(np.asarray(a, np.float32), -240, 240).astype(
        ml_dtypes.float8_e4m3)


# xs column -> local candidate index (-1 = pad/dead).  f=1 is the partial
# tile: columns 480:870 hold cands 480:870, 870:960 are dead; f>=2 tiles
# hold the remaining 2880 candidates.
_F1R = NSHARD - (NFC - 1) * FCH          # 390 real cands in the f=1 tile


def _col_to_loc():
    loc = np.full(NPAD, -1, np.int64)
    loc[0:FCH] = np.arange(FCH)
    loc[FCH:FCH + _F1R] = np.arange(FCH, FCH + _F1R)
    loc[2 * FCH:] = np.arange(FCH + _F1R, NSHARD)
    return loc


def _prep_inputs(X, uq, nq):
    """X: [N, 512] fp32; uq: unique query ids (len <= nq)."""
    kd = 2 * NKC * 128
    Qm = np.zeros((nq, kd), np.float32)
    nu = len(uq)
    Qm[:nu, :ND] = 2.0 * X[uq, :ND]
    Qm[:nu, ND] = S1
    Qm[:nu, ND + 1] = 1.0
    xq = np.ascontiguousarray(
        _f8(Qm).reshape(nq, 2 * NKC, 128).transpose(2, 1, 0))

    sqy = (X.astype(np.float64) ** 2).sum(1).astype(np.float32)
    bias_t = -(sqy - CENTER)
    b1 = _f8(bias_t / S1).astype(np.float32)
    b2 = _f8(bias_t - S1 * b1).astype(np.float32)

    loc = _col_to_loc()
    vm = loc >= 0
    per_core = []
    for ci in range(NCORES):
        sl = slice(ci * NSHARD, (ci + 1) * NSHARD)
        Xc = X[sl]
        Z = np.zeros((NPAD, kd), np.float32)
        Z[vm, :ND] = Xc[loc[vm], :ND]
        Z[vm, ND] = b1[sl][loc[vm]]
        Z[vm, ND + 1] = b2[sl][loc[vm]]
        Z[~vm, ND:ND + 2] = -240.0        # pad candidates rank last
        xs = np.ascontiguousarray(
            _f8(Z).reshape(NPAD, 2 * NKC, 128).transpose(2, 1, 0))
        per_core.append({"xq": xq, "xs": xs})
    return per_core


def _mine_chunkmax(in_maps, nqt, trace=False):
    from concourse.bass_utils import run_bass_kernel_spmd

    nc = _get_program(nqt)
    try:
        res = run_bass_kernel_spmd(nc, in_maps, list(range(NCORES)), trace=trace)
    except Exception:
        if not trace:
            raise
        res = run_bass_kernel_spmd(nc, in_maps, list(range(NCORES)), trace=False)
    _cache["last_result"] = res
    nblk = (nqt + QBLK - 1) // QBLK
    cores = []
    for i in range(NCORES):
        c = res.results[i]["cand"]                 # [nblk, 128, QBLK*240]
        c = c.reshape(nblk, 128, QBLK, NFC * NGRP).transpose(0, 2, 1, 3)
        cores.append(c.reshape(nblk * QBLK * 128, NFC * NGRP)[:nqt * QT])
    return np.concatenate(cores, axis=1)           # [nq, 1920]


def kernel(outlayer, c, train_ill, k):
    k = int(k)
    outlayer = np.asarray(outlayer, np.float32)
    train_ill = np.asarray(train_ill)
    X = np.ascontiguousarray(
        outlayer.transpose(1, 0, 2).reshape(N_, KD)).astype(np.float32)
    left = train_ill[:, 0].astype(np.int64)
    right = train_ill[:, 1].astype(np.int64)
    q_idx = np.concatenate([right, left])          # [2T]

    uq, inv = np.unique(q_idx, return_inverse=True)
    nqt = max(1, (len(uq) + QT - 1) // QT)
    nq = nqt * QT

    in_maps = _prep_inputs(X, uq, nq)
    cmu = _mine_chunkmax(
        in_maps, nqt, trace=bool(int(os.environ.get("KNN_TRACE", "0"))))
    cm = cmu[inv].astype(np.float32)               # [2T, 1920]

    # mask chunks the device never writes (f=1 groups >= F1G: stale SBUF)
    nch = NPAD // GW                               # 240 chunks per core
    ch = np.arange(nch)
    dead = ((ch // NGRP) == 1) & ((ch % NGRP) >= F1G)
    cm[:, np.tile(dead, NCORES)] = -np.inf

    # top-NSEL chunks per query -> candidate lists with known indices
    top_chunks = np.argpartition(-cm, NSEL, axis=1)[:, :NSEL]
    core = top_chunks // nch
    base_col = ((top_chunks % nch) // NGRP) * FCH \
        + ((top_chunks % nch) % NGRP) * GW
    loc = _col_to_loc()[
        base_col[:, :, None] + np.arange(GW)[None, None, :]]  # [2T,NSEL,16]
    valid = loc >= 0
    cand = np.where(valid, core[:, :, None] * NSHARD + np.maximum(loc, 0), 0)
    cand = cand.reshape(2 * T_, NSEL * GW)
    valid = valid.reshape(2 * T_, NSEL * GW)

    # exact rescore (fp32 gather/dot, fp64 assembly)
    nkeep = k + 1
    sq64 = (X.astype(np.float64) ** 2).sum(1)
    B_all = np.zeros((2 * T_, nkeep))
    for q0 in range(0, 2 * T_, 256):
        q1 = min(q0 + 256, 2 * T_)
        qv = X[q_idx[q0:q1]]                                   # [B, 512]
        cv = X[cand[q0:q1]]                                    # [B, C, 512]
        dot = np.matmul(cv, qv[:, :, None].astype(np.float32))[:, :, 0]
        d = (sq64[q_idx[q0:q1], None] + sq64[cand[q0:q1]]
             - 2.0 * dot.astype(np.float64))
        d = np.where(valid[q0:q1], d, np.inf)
        idx = np.argpartition(d, nkeep, axis=1)[:, :nkeep]
        g = X.astype(np.float64)[np.take_along_axis(cand[q0:q1], idx, axis=1)]
        dd = ((qv[:, None, :].astype(np.float64) - g) ** 2).sum(2)
        dd = np.where(np.take_along_axis(valid[q0:q1], idx, axis=1), dd, np.inf)
        B_all[q0:q1] = np.sort(dd, axis=1)
    B2 = B_all[:T_, 1:]            # right-query mining
    B1 = B_all[T_:, 1:]            # left-query mining

    X64 = X.astype(np.float64)
    D = ((X64[left] - X64[right]) ** 2).sum(1) + 1.0
    L1 = np.maximum(D[:, None] - B1, 0.0)
    L2 = np.maximum(D[:, None] - B2, 0.0)
    loss = (L1.mean() + L2.mean()) / 2.0
    return np.asarray(loss, dtype=np.float32)


# revision 31
# speedup vs baseline: 1.1042x; 1.0088x over previous
"""Trainium2 Bass kernel for CrossDecoder kNN-mining margin loss (fp8, v3).

Device mines approximate top candidates for 6000 queries over 30000
candidates (sharded 3750/core over 8 cores) via fp8 E4M3 DoubleRow matmuls:
score(q,j) = 2 q.y_j (ND data dims) + 32 b1_j + b2_j  ~=  2 q.y - (|y|^2-512).
The device emits per-16-candidate-chunk maxima; the host selects top chunks
(chunk POSITION identifies candidates), rescores exactly, computes the loss.

v3 changes (from 208us v2):
  - query dedup: only unique train_ill indices are mined (5438 -> 43 query
    tiles instead of 47), host scatters chunk maxima back.
  - scan rebalance: per query tile, DVE tensor_reduce's 1 PSUM tile (banks
    0-1) directly, ScalarE copies the other 3 tiles to SBUF fp16 TRANSPOSED
    ([w,g] -> [g,w]), so the DVE max tree runs entirely in 2x mode on
    contiguous stride-1 slices (no 1x tensor_reduce tail): 16->8->4->2->1
    pairwise tensor_tensor maxes.  DVE/j ~2.9us, Scalar/j ~2.8us, both under
    the PE's 3.5us (K=512), hiding the scan and freeing PSUM banks sooner
    (v2 stalled PE ~0.66us/j on DVE bank drains).
  - xs (candidates) DMA split into per-f-tile chunks so the first matmuls
    start before the full 2MB load lands; ~26 dummy warmup matmuls run
    during the DMA so the PE HAM clock-gate (1.2GHz cold) is released by
    the time real work starts.
"""

import os
import numpy as np
import ml_dtypes

M_, N_, D_, T_ = 2, 30000, 256, 3000
KD = M_ * D_                   # 512 contraction (data) dims
NCORES = 8
NSHARD = N_ // NCORES          # 3750
GW = 16                        # candidates per chunk
FCH = 480                      # candidate tile width (one PSUM bank, 30 groups)
NFC = 8                        # candidate tiles per core
NPAD = FCH * NFC               # 3840
NGRP = FCH // GW               # 30 chunk maxima per tile
S1 = 32.0                      # bias row 1 scale
CENTER = 512.0                 # |y|^2 centering (cancels in ranking)
QT = 128                       # queries per tile (PSUM partition dim)
QBLK = 4                       # query tiles per DMA block
XCH = 2                        # xs f-tiles per DMA chunk
F1W = 400                      # f=1 is a partial tile: 390 real cands + 10
F1G = F1W // GW                # pad, rounded to 25 chunk groups (saves 80
                               # matmul columns/pass vs padding to 480)

NKC = int(os.environ.get("KNN_NKC", "2"))   # k-tile pairs: 2 -> K=512 screen
ND = 128 * 2 * NKC - 2                      # data dims used for selection
NSEL = int(os.environ.get("KNN_NSEL", "48" if NKC == 2 else "96"))

_cache = {}


def _build_program(nqt):
    import concourse.bass as bass
    import concourse.tile as tile
    from concourse import bacc, mybir

    dt = mybir.dt
    nc = bacc.Bacc(
        "TRN2", target_bir_lowering=False, debug=False, num_devices=NCORES
    )

    nq = nqt * QT
    nblk = (nqt + QBLK - 1) // QBLK
    xq_d = nc.dram_tensor("xq", [128, 2 * NKC, nq], dt.float8e4,
                          kind="ExternalInput")
    xs_d = nc.dram_tensor("xs", [128, 2 * NKC, NPAD], dt.float8e4,
                          kind="ExternalInput")
    cand_d = nc.dram_tensor("cand", [nblk, 128, QBLK * NFC * NGRP], dt.float16,
                            kind="ExternalOutput")

    DR = mybir.MatmulPerfMode.DoubleRow

    with tile.TileContext(nc) as tc:
        with (
            tc.tile_pool(name="resident", bufs=1) as res_pool,
            tc.tile_pool(name="xq", bufs=2) as xq_pool,
            tc.tile_pool(name="cand", bufs=2) as cand_pool,
            tc.tile_pool(name="scr", bufs=2) as scr_pool,
            tc.tile_pool(name="psum", bufs=4, space=bass.MemorySpace.PSUM) as psum_pool,
        ):
            # --- resident candidates: one SBUF tile per DMA chunk, so the
            # first matmuls' dependencies resolve as soon as their own chunk
            # lands (a single tile would gate on the full 2MB load).
            xs_sbs = [res_pool.tile([128, 2 * NKC, XCH * FCH], dt.float8e4,
                                    tag=f"xs{ci}", name=f"xs{ci}")
                      for ci in range(NFC // XCH)]

            def xs_slice(kc, f):
                t = xs_sbs[f // XCH]
                fo = f % XCH
                w = F1W if f == 1 else FCH
                return t[:, 2 * kc:2 * kc + 2, fo * FCH:fo * FCH + w]

            xq_tiles = {}

            def xq_dma(blk):
                q0 = blk * QBLK * QT
                bqt = min(QBLK, nqt - blk * QBLK)
                t = xq_pool.tile([128, 2 * NKC, bqt * QT], dt.float8e4,
                                 tag="xq", name="xq_sb")
                # block 0 triggers from the (idle) Scalar queue, in parallel
                # with the sync queue's xs chunk triggers
                eng = nc.scalar if blk == 0 else nc.sync
                eng.dma_start(out=t[:, :, :],
                              in_=xq_d[:, :, q0:q0 + bqt * QT])
                xq_tiles[blk] = t

            def xs_dma(ci):
                c0 = ci * XCH * FCH
                nc.sync.dma_start(out=xs_sbs[ci][:, :, :],
                                  in_=xs_d[:, :, c0:c0 + XCH * FCH])

            xs_dma(0)
            xq_dma(0)
            for ci in range(1, NFC // XCH):
                xs_dma(ci)

            def emit_tree(scr, cand_out, ntile=3):
                """DVE max tree over ntile Scalar-copied tiles: 16->8->4->2
                in 2x mode, final 2->1 a strided 1x tensor_tensor.
                scr: [128, ntile, 2, NGRP, GW]; cand_out: [128, 2*ntile, NGRP]
                """
                t1 = scr_pool.tile([128, ntile, 2, NGRP, 8], dt.float16,
                                   tag=f"t1_{ntile}")
                nc.vector.tensor_tensor(
                    t1[:, :, :, :, :], scr[:, :, :, :, 0:8],
                    scr[:, :, :, :, 8:16], mybir.AluOpType.max)
                t2 = scr_pool.tile([128, ntile, 2, NGRP, 4], dt.float16,
                                   tag=f"t2_{ntile}")
                nc.vector.tensor_tensor(
                    t2[:, :, :, :, :], t1[:, :, :, :, 0:4],
                    t1[:, :, :, :, 4:8], mybir.AluOpType.max)
                t3 = scr_pool.tile([128, ntile, 2, NGRP, 2], dt.float16,
                                   tag=f"t3_{ntile}")
                nc.vector.tensor_tensor(
                    t3[:, :, :, :, :], t2[:, :, :, :, 0:2],
                    t2[:, :, :, :, 2:4], mybir.AluOpType.max)
                nc.vector.tensor_tensor(
                    cand_out.rearrange("p (t a) g -> p t a g", t=ntile),
                    t3[:, :, :, :, 0], t3[:, :, :, :, 1],
                    mybir.AluOpType.max)

            # tree/output emission is deferred one query tile so the DVE
            # queue runs [reduce_j, tree_{j-1}, reduce_{j+1}, ...]: the
            # reduce that frees PSUM banks 0-1 for the next tile's matmuls
            # is never stuck behind a tree waiting on Scalar copies.
            pending_tree = None
            pending_cand = None

            for blk in range(nblk):
                bqt = min(QBLK, nqt - blk * QBLK)
                if blk + 1 < nblk:
                    xq_dma(blk + 1)
                xq_sb = xq_tiles.pop(blk)
                cand_sb = cand_pool.tile([128, bqt, NFC, NGRP], dt.float16,
                                         tag="cand")
                for j in range(bqt):
                    ps = [psum_pool.tile([128, 2, NGRP, GW], dt.float32,
                                         tag="ps", name=f"ps{p}",
                                         padded_shape=[None, None, 32, None])
                          for p in range(4)]

                    def mm(kc, f):
                        ng = F1G if f == 1 else NGRP
                        nc.tensor.matmul(
                            ps[f // 2][:, f % 2, 0:ng, :],
                            lhsT=xq_sb[:, 2 * kc:2 * kc + 2,
                                       j * QT:(j + 1) * QT],
                            rhs=xs_slice(kc, f),
                            start=(kc == 0),
                            stop=(kc == NKC - 1),
                            perf_mode=DR,
                        )
                    # f-major, kc-minor: each PSUM pair p completes at MM
                    # #(4p+4), so its consumer (reduce / Scalar copy) starts
                    # as early as possible and frees banks for the next tile.
                    # The two stationaries ping-pong the PE weight buffers;
                    # LDWEIGHTS (135ns) hides under each matmul (203ns).
                    # Ordinary tiles: DVE tensor_reduce's p0, Scalar copies
                    # p1..p3.  The LAST tile flips roles (Scalar copies
                    # p0..p2 with per-pair trees, DVE reduces p3 at the end)
                    # so only a ~1.1us reduce trails the final matmul.
                    last = (blk == nblk - 1) and (j == bqt - 1)
                    rset = {0, 3} if last else {0}  # the DVE-reduced pairs
                    scr = scr_pool.tile([128, 3, 2, NGRP, GW], dt.float16,
                                        tag="scr")
                    si = 0
                    for f in range(NFC):
                        for kc in range(NKC):
                            mm(kc, f)
                        if f % 2 == 1:
                            p = f // 2
                            if p in rset:
                                nc.vector.tensor_reduce(
                                    cand_sb[:, j, 2 * p:2 * p + 2, :],
                                    ps[p][:, :, :, :],
                                    axis=mybir.AxisListType.X,
                                    op=mybir.AluOpType.max,
                                )
                            elif last:
                                nc.scalar.activation(
                                    scr[:, si, :, :, :], ps[p][:, :, :, :],
                                    mybir.ActivationFunctionType.Copy,
                                )
                                emit_tree(scr[:, si:si + 1, :, :, :],
                                          cand_sb[:, j, 2 * p:2 * p + 2, :],
                                          ntile=1)
                                si += 1
                    if pending_tree is not None:
                        emit_tree(*pending_tree)
                        pending_tree = None
                    if not last:
                        # tiles p1..p3: ScalarE contiguous copy PSUM->SBUF
                        for p in range(1, 4):
                            nc.scalar.activation(
                                scr[:, p - 1, :, :, :], ps[p][:, :, :, :],
                                mybir.ActivationFunctionType.Copy,
                            )
                        pending_tree = (scr, cand_sb[:, j, 2:8, :])
                    if pending_cand is not None:
                        nc.scalar.dma_start(out=pending_cand[0],
                                            in_=pending_cand[1])
                        pending_cand = None
                # output DMA on the Scalar queue (HWDGE) so the sync queue
                # never blocks on a cand-ready wait; deferred past the next
                # tile's copies so the wait is already satisfied.
                pending_cand = (cand_d[blk, :, :bqt * NFC * NGRP],
                                cand_sb[:, :, :, :])
            if pending_tree is not None:
                emit_tree(*pending_tree)
            if pending_cand is not None:
                nc.scalar.dma_start(out=pending_cand[0], in_=pending_cand[1])

    nc.compile()
    return nc


def _get_program(nqt):
    key = ("nc", nqt, NKC)
    if key not in _cache:
        _cache[key] = _build_program(nqt)
    return _cache[key]


def _f8(a):
    return np.clip(np.asarray(a, np.float32), -240, 240).astype(
        ml_dtypes.float8_e4m3)


# xs column -> local candidate index (-1 = pad/dead).  f=1 is the partial
# tile: columns 480:870 hold cands 480:870, 870:960 are dead; f>=2 tiles
# hold the remaining 2880 candidates.
_F1R = NSHARD - (NFC - 1) * FCH          # 390 real cands in the f=1 tile


def _col_to_loc():
    loc = np.full(NPAD, -1, np.int64)
    loc[0:FCH] = np.arange(FCH)
    loc[FCH:FCH + _F1R] = np.arange(FCH, FCH + _F1R)
    loc[2 * FCH:] = np.arange(FCH + _F1R, NSHARD)
    return loc


def _prep_inputs(X, uq, nq):
    """X: [N, 512] fp32; uq: unique query ids (len <= nq)."""
    kd = 2 * NKC * 128
    Qm = np.zeros((nq, kd), np.float32)
    nu = len(uq)
    Qm[:nu, :ND] = 2.0 * X[uq, :ND]
    Qm[:nu, ND] = S1
    Qm[:nu, ND + 1] = 1.0
    xq = np.ascontiguousarray(
        _f8(Qm).reshape(nq, 2 * NKC, 128).transpose(2, 1, 0))

    sqy = (X.astype(np.float64) ** 2).sum(1).astype(np.float32)
    bias_t = -(sqy - CENTER)
    b1 = _f8(bias_t / S1).astype(np.float32)
    b2 = _f8(bias_t - S1 * b1).astype(np.float32)

    loc = _col_to_loc()
    vm = loc >= 0
    per_core = []
    for ci in range(NCORES):
        sl = slice(ci * NSHARD, (ci + 1) * NSHARD)
        Xc = X[sl]
        Z = np.zeros((NPAD, kd), np.float32)
        Z[vm, :ND] = Xc[loc[vm], :ND]
        Z[vm, ND] = b1[sl][loc[vm]]
        Z[vm, ND + 1] = b2[sl][loc[vm]]
        Z[~vm, ND:ND + 2] = -240.0        # pad candidates rank last
        xs = np.ascontiguousarray(
            _f8(Z).reshape(NPAD, 2 * NKC, 128).transpose(2, 1, 0))
        per_core.append({"xq": xq, "xs": xs})
    return per_core


def _mine_chunkmax(in_maps, nqt, trace=False):
    from concourse.bass_utils import run_bass_kernel_spmd

    nc = _get_program(nqt)
    try:
        res = run_bass_kernel_spmd(nc, in_maps, list(range(NCORES)), trace=trace)
    except Exception:
        if not trace:
            raise
        res = run_bass_kernel_spmd(nc, in_maps, list(range(NCORES)), trace=False)
    _cache["last_result"] = res
    nblk = (nqt + QBLK - 1) // QBLK
    cores = []
    for i in range(NCORES):
        c = res.results[i]["cand"]                 # [nblk, 128, QBLK*240]
        c = c.reshape(nblk, 128, QBLK, NFC * NGRP).transpose(0, 2, 1, 3)
        cores.append(c.reshape(nblk * QBLK * 128, NFC * NGRP)[:nqt * QT])
    return np.concatenate(cores, axis=1)           # [nq, 1920]


def kernel(outlayer, c, train_ill, k):
    k = int(k)
    outlayer = np.asarray(outlayer, np.float32)
    train_ill = np.asarray(train_ill)
    X = np.ascontiguousarray(
        outlayer.transpose(1, 0, 2).reshape(N_, KD)).astype(np.float32)
    left = train_ill[:, 0].astype(np.int64)
    right = train_ill[:, 1].astype(np.int64)
    q_idx = np.concatenate([right, left])          # [2T]

    uq, inv = np.unique(q_idx, return_inverse=True)
    nqt = max(1, (len(uq) + QT - 1) // QT)
    nq = nqt * QT

    in_maps = _prep_inputs(X, uq, nq)
    cmu = _mine_chunkmax(
        in_maps, nqt, trace=bool(int(os.environ.get("KNN_TRACE", "0"))))
    cm = cmu[inv].astype(np.float32)               # [2T, 1920]

    # mask chunks the device never writes (f=1 groups >= F1G: stale SBUF)
    nch = NPAD // GW                               # 240 chunks per core
    ch = np.arange(nch)
    dead = ((ch // NGRP) == 1) & ((ch % NGRP) >= F1G)
    cm[:, np.tile(dead, NCORES)] = -np.inf

    # top-NSEL chunks per query -> candidate lists with known indices
    top_chunks = np.argpartition(-cm, NSEL, axis=1)[:, :NSEL]
    core = top_chunks // nch
    base_col = ((top_chunks % nch) // NGRP) * FCH \
        + ((top_chunks % nch) % NGRP) * GW
    loc = _col_to_loc()[
        base_col[:, :, None] + np.arange(GW)[None, None, :]]  # [2T,NSEL,16]
    valid = loc >= 0
    cand = np.where(valid, core[:, :, None] * NSHARD + np.maximum(loc, 0), 0)
    cand = cand.reshape(2 * T_, NSEL * GW)
    valid = valid.reshape(2 * T_, NSEL * GW)

    # exact rescore (fp32 gather/dot, fp64 assembly)
    nkeep = k + 1
    sq64 = (X.astype(np.float64) ** 2).sum(1)
    B_all = np.zeros((2 * T_, nkeep))
    for q0 in range(0, 2 * T_, 256):
        q1 = min(q0 + 256, 2 * T_)
        qv = X[q_idx[q0:q1]]                                   # [B, 512]
        cv = X[cand[q0:q1]]                                    # [B, C, 512]
        dot = np.matmul(cv, qv[:, :, None].astype(np.float32))[:, :, 0]
        d = (sq64[q_idx[q0:q1], None] + sq64[cand[q0:q1]]
             - 2.0 * dot.astype(np.float64))
        d = np.where(valid[q0:q1], d, np.inf)
        idx = np.argpartition(d, nkeep, axis=1)[:, :nkeep]
        g = X.astype(np.float64)[np.take_along_axis(cand[q0:q1], idx, axis=1)]
        dd = ((qv[:, None, :].astype(np.float64) - g) ** 2).sum(2)
        dd = np.where(np.take_along_axis(valid[q0:q1], idx, axis=1), dd, np.inf)
        B_all[q0:q1] = np.sort(dd, axis=1)
    B2 = B_all[:T_, 1:]            # right-query mining
    B1 = B_all[T_:, 1:]            # left-query mining

    X64 = X.astype(np.float64)
    D = ((X64[left] - X64[right]) ** 2).sum(1) + 1.0
    L1 = np.maximum(D[:, None] - B1, 0.0)
    L2 = np.maximum(D[:, None] - B2, 0.0)
    loss = (L1.mean() + L2.mean()) / 2.0
    return np.asarray(loss, dtype=np.float32)
